# revision 3
# baseline (speedup 1.0000x reference)
"""Trainium2 Bass kernel for the entity-assignment loss.

Math: per sample b, C[i,j] = mean_d (yt[b,i,d]-yp[b,j,d])^2.
loss = mean_b ( min_perm sum_i C[i, perm(i)] / 8 ).

Since each permutation uses every row i and every column j exactly once,
  sum_i C[i, perm(i)] = (nt + np - 2 * sum_i dot(i, perm(i))) / 64
so min over perms only needs MAX over perms of the dot sums G[i,j]
(computed with a fp16 2x DVE multiply + fold tree per 128-sample chunk).

The max-assignment is evaluated with a fully PACKED combinatorial DP:
every level stores only the valid k-subsets (28/56/70/56 ranks), and each
level is one GpSimd ap_gather (static index table pairing source values
with their G addend), one contiguous DVE tensor_tensor add, and one DVE
segmented tensor_reduce max:
  Q2[{x,y}]  = best of cols {0,1} on rows {x,y}     (2x28 ranks, 2 slots)
  P3[{x,y,z}]= best of cols {0,1,2}                  (56 ranks, 3 slots)
  S3 / Qs2   = same from the suffix side, cols {5,6,7}
  dp4        = P3 + col 3                            (70 ranks, 4 slots)
  dp5        = dp4 + col 4                           (56 ranks, 5 slots)
  ans        = max_r dp5[r] + S3[complement(r)]      (one 64-idx gather)
All stages of one chunk live in a single gatherable "combo" tile.

Sharding: pure data parallelism, 256 samples per core across 8 cores
(2 partition-chunks of 128 samples), chunk pipelines interleaved so Pool
gathers hide under DVE compute of the other chunk.
"""

import sys

if "/opt/trn_rl_repo" not in sys.path:
    sys.path.insert(0, "/opt/trn_rl_repo")

import numpy as np

B, N, D = 2048, 8, 64
N_CORES = 8
B_LOC = B // N_CORES        # 256 samples per core
NT = B_LOC // 128           # 2 partition chunks of 128 samples

TRACE = False
_CACHE = {}

# region offsets inside the per-chunk combo tile [128, 360]
GOFF, Q2OFF, QS2OFF, P3OFF, S3OFF, DP4OFF, DP5OFF = 0, 64, 92, 120, 176, 232, 302
COMBO_W = 360


def _idx_tables():
    import itertools

    pairs = list(itertools.combinations(range(N), 2))
    trips = list(itertools.combinations(range(N), 3))
    quads = list(itertools.combinations(range(N), 4))
    quints = list(itertools.combinations(range(N), 5))
    r2 = {p: i for i, p in enumerate(pairs)}
    r3 = {t: i for i, t in enumerate(trips)}
    r4 = {q: i for i, q in enumerate(quads)}

    h1, h2 = [], []
    for cols in ((0, 1), (6, 7)):
        for (x, y) in pairs:
            for (u, v) in ((x, y), (y, x)):
                h1.append(GOFF + u * 8 + cols[0])
                h2.append(GOFF + v * 8 + cols[1])
    g_pair = h1 + h2                     # 224
    h1, h2 = [], []
    for tbloff, col in ((Q2OFF, 2), (QS2OFF, 5)):
        for T in trips:
            for i in T:
                rest = tuple(x for x in T if x != i)
                h1.append(tbloff + r2[rest])
                h2.append(GOFF + i * 8 + col)
    g_p3s3 = h1 + h2                     # 672
    h1, h2 = [], []
    for S in quads:
        for i in S:
            rest = tuple(x for x in S if x != i)
            h1.append(P3OFF + r3[rest])
            h2.append(GOFF + i * 8 + 3)
    g_l3 = h1 + h2                       # 560
    h1, h2 = [], []
    for S in quints:
        for i in S:
            rest = tuple(x for x in S if x != i)
            h1.append(DP4OFF + r4[rest])
            h2.append(GOFF + i * 8 + 4)
    g_l4 = h1 + h2                       # 560
    g_fin = []
    for S in quints:
        comp = tuple(x for x in range(N) if x not in S)
        g_fin.append(S3OFF + r3[comp])
    g_fin += [0] * 8                     # 64

    def wrap(lst):
        ncol = len(lst) // 16
        w = np.empty((128, ncol), dtype=np.int16)
        for p in range(128):
            for s in range(ncol):
                w[p, s] = lst[s * 16 + (p % 16)]
        return w

    return np.concatenate(
        [wrap(l) for l in (g_pair, g_p3s3, g_l3, g_l4, g_fin)], axis=1)


# idx column ranges within the [128, 130] idx tile
IC_PAIR = (0, 14)
IC_P3S3 = (14, 56)
IC_L3 = (56, 91)
IC_L4 = (91, 126)
IC_FIN = (126, 130)


def _build():
    import concourse.bacc as bacc
    import concourse.mybir as mybir
    from concourse.tile import TileContext

    f32 = mybir.dt.float32
    f16 = mybir.dt.float16
    i16 = mybir.dt.int16
    Alu = mybir.AluOpType
    Act = mybir.ActivationFunctionType
    AX = mybir.AxisListType.X

    nc = bacc.Bacc("TRN2", target_bir_lowering=False, debug=False)
    yt_d = nc.declare_dram_parameter("yt", [B_LOC, N * D], f32, isOutput=False)
    yp_d = nc.declare_dram_parameter("yp", [B_LOC, N * D], f32, isOutput=False)
    ix_d = nc.declare_dram_parameter("idx", [128, 130], i16, isOutput=False)
    out_d = nc.declare_dram_parameter("out", [128, NT], f32, isOutput=True)

    with TileContext(nc) as tc:
        with (
            tc.tile_pool(name="io", bufs=1) as io_pool,
            tc.tile_pool(name="work", bufs=1) as work_pool,
            tc.tile_pool(name="res", bufs=1) as res_pool,
        ):
            # ---- tiles ----
            ytf = [io_pool.tile([128, N * D], f32, tag=f"ytf{c}", name=f"ytf{c}")
                   for c in range(NT)]
            ypf = [io_pool.tile([128, N * D], f32, tag=f"ypf{c}", name=f"ypf{c}")
                   for c in range(NT)]
            idx = io_pool.tile([128, 130], i16, tag="idx", name="idx")
            yth = [work_pool.tile([128, N * D], f16, tag=f"yth{c}", name=f"yth{c}")
                   for c in range(NT)]
            yph = [work_pool.tile([128, N * D], f16, tag=f"yph{c}", name=f"yph{c}")
                   for c in range(NT)]
            prod = work_pool.tile([128, N * N * D], f16, tag="prod", name="prod")
            half = work_pool.tile([128, N * N * D // 2], f16, tag="half", name="half")
            quart = work_pool.tile([128, N * N * D // 4], f16, tag="quart",
                                   name="quart")
            eighth = work_pool.tile([128, N * N * D // 8], f16, tag="eighth",
                                    name="eighth")
            combo = [res_pool.tile([128, COMBO_W], f32, tag=f"combo{c}",
                                   name=f"combo{c}") for c in range(NT)]
            gpair = [res_pool.tile([128, 224], f32, tag=f"gpair{c}",
                                   name=f"gpair{c}") for c in range(NT)]
            gp3s3 = [res_pool.tile([128, 672], f32, tag=f"gp3s3{c}",
                                   name=f"gp3s3{c}") for c in range(NT)]
            gl3 = [res_pool.tile([128, 560], f32, tag=f"gl3{c}", name=f"gl3{c}")
                   for c in range(NT)]
            gl4 = [res_pool.tile([128, 560], f32, tag=f"gl4{c}", name=f"gl4{c}")
                   for c in range(NT)]
            gfin = [res_pool.tile([128, 64], f32, tag=f"gfin{c}", name=f"gfin{c}")
                    for c in range(NT)]
            spair = [work_pool.tile([128, 112], f32, tag=f"spair{c}",
                                    name=f"spair{c}") for c in range(NT)]
            sp3s3 = [work_pool.tile([128, 336], f32, tag=f"sp3s3{c}",
                                    name=f"sp3s3{c}") for c in range(NT)]
            sl3 = [work_pool.tile([128, 280], f32, tag=f"sl3{c}", name=f"sl3{c}")
                   for c in range(NT)]
            sl4 = [work_pool.tile([128, 280], f32, tag=f"sl4{c}", name=f"sl4{c}")
                   for c in range(NT)]
            cm = [work_pool.tile([128, 56], f32, tag=f"cm{c}", name=f"cm{c}")
                  for c in range(NT)]
            nt_t = [res_pool.tile([128, 1], f32, tag=f"nt{c}", name=f"nt{c}")
                    for c in range(NT)]
            npt = [res_pool.tile([128, 1], f32, tag=f"npt{c}", name=f"npt{c}")
                   for c in range(NT)]
            sq = work_pool.tile([128, N * D], f32, tag="sq", name="sq")
            s_all = res_pool.tile([128, NT], f32, tag="s_all", name="s_all")
            dmax = res_pool.tile([128, NT], f32, tag="dmax", name="dmax")
            loss_t = res_pool.tile([128, NT], f32, tag="loss", name="loss")

            # ---- input DMAs ----
            for c in range(NT):
                nc.sync.dma_start(out=ytf[c][:, :], in_=yt_d[c * 128:(c + 1) * 128, :])
                nc.sync.dma_start(out=ypf[c][:, :], in_=yp_d[c * 128:(c + 1) * 128, :])
            nc.gpsimd.dma_start(out=idx[:, :], in_=ix_d[:, :])

            # ---- Scalar: casts then norms ----
            for c in range(NT):
                nc.scalar.activation(out=yth[c][:, :], in_=ytf[c][:, :],
                                     func=Act.Identity)
                nc.scalar.activation(out=yph[c][:, :], in_=ypf[c][:, :],
                                     func=Act.Identity)
            for c in range(NT):
                nc.scalar.activation(out=sq[:, :], in_=ytf[c][:, :], func=Act.Square,
                                     accum_out=nt_t[c][:, 0:1])
                nc.scalar.activation(out=sq[:, :], in_=ypf[c][:, :], func=Act.Square,
                                     accum_out=npt[c][:, 0:1])

            # ---- helpers ----
            def g_chain(c):
                """combo[c][:, 0:64] = G[p, i*8+j] = dot(yt_i, yp_j)."""
                yt_b = yth[c].rearrange("p (i d) -> p i d", d=D).unsqueeze(2) \
                    .broadcast_to([128, N, N, D])
                yp_b = yph[c].rearrange("p (j d) -> p j d", d=D).unsqueeze(1) \
                    .broadcast_to([128, N, N, D])
                nc.vector.tensor_tensor(
                    out=prod.rearrange("p (i j d) -> p i j d", j=N, d=D),
                    in0=yt_b, in1=yp_b, op=Alu.mult)
                pv = prod.rearrange("p (q d) -> p q d", d=D)
                hv = half.rearrange("p (q d) -> p q d", d=D // 2)
                nc.vector.tensor_tensor(
                    out=hv, in0=pv[:, :, 0:D // 2], in1=pv[:, :, D // 2:D], op=Alu.add)
                qv = quart.rearrange("p (q d) -> p q d", d=D // 4)
                nc.vector.tensor_tensor(
                    out=qv, in0=hv[:, :, 0:D // 4], in1=hv[:, :, D // 4:D // 2],
                    op=Alu.add)
                ev = eighth.rearrange("p (q d) -> p q d", d=D // 8)
                nc.vector.tensor_tensor(
                    out=ev, in0=qv[:, :, 0:D // 8], in1=qv[:, :, D // 8:D // 4],
                    op=Alu.add)
                nc.vector.tensor_reduce(
                    out=combo[c][:, GOFF:GOFF + 64], in_=ev, axis=AX, op=Alu.add)

            def gather(c, out_t, cols, n):
                nc.gpsimd.ap_gather(
                    out_ap=out_t[c][:, :], in_ap=combo[c][:, :],
                    idxs_ap=idx[:, cols[0]:cols[1]], channels=128,
                    num_elems=COMBO_W, d=1, num_idxs=n)

            def stage(c, g_t, s_t, half_n, groups, slots, ooff, on):
                """sums = g[:half]+g[half:]; combo[ooff:ooff+on] = segmented max."""
                nc.vector.tensor_tensor(
                    out=s_t[c][:, :], in0=g_t[c][:, 0:half_n],
                    in1=g_t[c][:, half_n:2 * half_n], op=Alu.add)
                nc.vector.tensor_reduce(
                    out=combo[c][:, ooff:ooff + on],
                    in_=s_t[c].rearrange("p (g s) -> p g s", s=slots),
                    axis=AX, op=Alu.max)

            def fin(c):
                nc.vector.tensor_tensor(
                    out=cm[c][:, :], in0=combo[c][:, DP5OFF:DP5OFF + 56],
                    in1=gfin[c][:, 0:56], op=Alu.add)
                nc.vector.tensor_reduce(
                    out=dmax[:, c:c + 1], in_=cm[c][:, :], axis=AX, op=Alu.max)
                nc.vector.tensor_add(s_all[:, c:c + 1], nt_t[c][:, 0:1],
                                     npt[c][:, 0:1])
                nc.vector.scalar_tensor_tensor(
                    out=loss_t[:, c:c + 1], in0=dmax[:, c:c + 1], scalar=-2.0,
                    in1=s_all[:, c:c + 1], op0=Alu.mult, op1=Alu.add)

            # ---- schedule (interleaved so Pool gathers hide under DVE) ----
            g_chain(0)
            gather(0, gpair, IC_PAIR, 224)                        # Pool
            g_chain(1)
            stage(0, gpair, spair, 112, 56, 2, Q2OFF, 56)         # DVE Q2/Qs2 c0
            gather(0, gp3s3, IC_P3S3, 672)                        # Pool
            gather(1, gpair, IC_PAIR, 224)                        # Pool
            stage(0, gp3s3, sp3s3, 336, 112, 3, P3OFF, 112)       # DVE P3/S3 c0
            gather(0, gl3, IC_L3, 560)                            # Pool
            stage(1, gpair, spair, 112, 56, 2, Q2OFF, 56)         # DVE Q2/Qs2 c1
            gather(1, gp3s3, IC_P3S3, 672)                        # Pool
            stage(0, gl3, sl3, 280, 70, 4, DP4OFF, 70)            # DVE dp4 c0
            gather(0, gl4, IC_L4, 560)                            # Pool
            stage(1, gp3s3, sp3s3, 336, 112, 3, P3OFF, 112)       # DVE P3/S3 c1
            gather(1, gl3, IC_L3, 560)                            # Pool
            stage(0, gl4, sl4, 280, 56, 5, DP5OFF, 56)            # DVE dp5 c0
            gather(0, gfin, IC_FIN, 64)                           # Pool
            stage(1, gl3, sl3, 280, 70, 4, DP4OFF, 70)            # DVE dp4 c1
            gather(1, gl4, IC_L4, 560)                            # Pool
            fin(0)                                                # DVE
            stage(1, gl4, sl4, 280, 56, 5, DP5OFF, 56)            # DVE dp5 c1
            gather(1, gfin, IC_FIN, 64)                           # Pool
            fin(1)                                                # DVE
            nc.sync.dma_start(out=out_d[:, :], in_=loss_t[:, :])
    nc.compile()
    return nc


def kernel(y_true: np.ndarray, y_pred: np.ndarray) -> np.ndarray:
    from concourse.bass_utils import run_bass_kernel_spmd

    if "nc" not in _CACHE:
        _CACHE["nc"] = _build()
    nc = _CACHE["nc"]

    yt = np.ascontiguousarray(np.asarray(y_true, dtype=np.float32)).reshape(B, N * D)
    yp = np.ascontiguousarray(np.asarray(y_pred, dtype=np.float32)).reshape(B, N * D)
    idx = _idx_tables()

    in_maps = [
        {
            "yt": np.ascontiguousarray(yt[c * B_LOC:(c + 1) * B_LOC]),
            "yp": np.ascontiguousarray(yp[c * B_LOC:(c + 1) * B_LOC]),
            "idx": idx,
        }
        for c in range(N_CORES)
    ]
    res = run_bass_kernel_spmd(nc, in_maps, list(range(N_CORES)), trace=TRACE)
    _CACHE["last_results"] = res
    vals = np.concatenate([np.asarray(r["out"], dtype=np.float64).reshape(-1)
                           for r in res.results])
    loss = vals.mean() / (D * N)
    return np.float32(loss)
